# revision 1
# baseline (speedup 1.0000x reference)
"""Trainium2 Bass kernel for the JaCDE dense-MLP vector-field problem.

Math restructuring: the reference materializes d_outer[b,i,j] = dtanh[b,i] *
wout[i,j] * drelu[b,j] and contracts it (O(B*H^3)).  Algebraically the whole
computation is 9 applications of the per-sample linear operator
    M v = dtanh ∘ (wout @ (drelu ∘ (wh @ v)))
applied to jx = dtanh ∘ (wout @ (drelu ∘ (wx @ xdot))), with
    h_dot = sum_{k=0..8} M^k jx
which is O(B*H^2) matmuls.  All activations live transposed [H, B_local] so
batch is the matmul free dim (N=256) and H splits into 2 partition tiles.

Sharding: pure data parallel, batch 2048 -> 8 cores x 256.
"""

import numpy as np

import concourse.tile as tile
from concourse import bacc, mybir
from concourse.bass_utils import run_bass_kernel_spmd

B, H, IN = 2048, 256, 64
K_TERMS = 8
N_CORES = 8
BL = B // N_CORES  # 256 batch rows per core
HH = H // 2  # 128, H partition halves

f32 = mybir.dt.float32
# float32r: PE reads fp32 operands through the fast (1 cycle/row for N>=256)
# path with mantissa rounding; measured max-rel-err ~1.6e-4 on HW, same
# envelope as the plain fp32 PE path which runs 4x slower.
MM_DT = mybir.dt.float32r
N_WARMUP_MM = 16  # ~3us of PE work to lift the HAM clock gate before real MMs

_ALU = mybir.AluOpType
_ACT = mybir.ActivationFunctionType


def _build(repeat=1, loop=0):
    nc = bacc.Bacc(None, target_bir_lowering=False)

    # Per-core inputs (already transposed / sharded by the host wrapper).
    hT = nc.dram_tensor("hT", [H, BL], f32, kind="ExternalInput")
    xT = nc.dram_tensor("xT", [IN, BL], f32, kind="ExternalInput")
    xdT = nc.dram_tensor("xdT", [IN, BL], MM_DT, kind="ExternalInput")
    wxT = nc.dram_tensor("wxT", [IN, H], f32, kind="ExternalInput")
    whT = nc.dram_tensor("whT", [H, H], f32, kind="ExternalInput")
    woT = nc.dram_tensor("woT", [H, H], MM_DT, kind="ExternalInput")
    b0c = nc.dram_tensor("b0c", [HH, 2], f32, kind="ExternalInput")
    b1c = nc.dram_tensor("b1c", [HH, 2], f32, kind="ExternalInput")
    hdT = nc.dram_tensor("hdT", [H, BL], f32, kind="ExternalOutput")

    with tile.TileContext(nc) as tc:
        with (
            tc.tile_pool(name="wpool", bufs=1) as wpool,
            tc.tile_pool(name="apool", bufs=1) as apool,
            tc.tile_pool(name="rot", bufs=4) as rot,
            tc.tile_pool(name="tgp", bufs=3) as tgp,
            tc.tile_pool(name="ps", bufs=8, space="PSUM") as ps,
        ):
            # ---- weights / inputs to SBUF ----
            whF = [wpool.tile([HH, H], f32, tag=f"whF{k}", name=f"whF{k}") for k in range(2)]
            wh_sb = [wpool.tile([HH, H], MM_DT, tag=f"wh{k}", name=f"wh{k}") for k in range(2)]
            wo_sb = [wpool.tile([HH, H], MM_DT, tag=f"wo{k}", name=f"wo{k}") for k in range(2)]
            wxF = wpool.tile([IN, H], f32, tag="wxF")
            wx_sb = wpool.tile([IN, H], MM_DT, tag="wx")
            h_sb = [wpool.tile([HH, BL], f32, tag=f"h{k}", name=f"h{k}") for k in range(2)]
            x_sb = wpool.tile([IN, BL], f32, tag="x")
            xd_sb = wpool.tile([IN, BL], MM_DT, tag="xd")
            b0_sb = wpool.tile([HH, 2], f32, tag="b0")
            b1_sb = wpool.tile([HH, 2], f32, tag="b1")
            for k in range(2):
                nc.sync.dma_start(whF[k][:], whT[k * HH : (k + 1) * HH, :])
                nc.sync.dma_start(wo_sb[k][:], woT[k * HH : (k + 1) * HH, :])
                nc.sync.dma_start(h_sb[k][:], hT[k * HH : (k + 1) * HH, :])
                # rounded copies for the fast f32r loop matmuls
                nc.vector.tensor_copy(wh_sb[k][:], whF[k][:])
            nc.sync.dma_start(wxF[:], wxT[:])
            nc.vector.tensor_copy(wx_sb[:], wxF[:])
            nc.sync.dma_start(x_sb[:], xT[:])
            nc.sync.dma_start(xd_sb[:], xdT[:])
            nc.sync.dma_start(b0_sb[:], b0c[:])
            nc.sync.dma_start(b1_sb[:], b1c[:])

            # ---- PE warmup: dummy matmuls with no data deps so the HAM
            # clock gate opens while the input DMAs are still in flight ----
            if N_WARMUP_MM:
                wu_w = wpool.tile([HH, HH], MM_DT, tag="wu_w")
                wu_v = wpool.tile([HH, BL], MM_DT, tag="wu_v")
                nc.vector.memset(wu_w[:].bitcast(f32), 0.0)
                nc.vector.memset(wu_v[:].bitcast(f32), 0.0)
                wu_p = ps.tile([HH, BL], f32, tag="ps")
                for _ in range(N_WARMUP_MM):
                    nc.tensor.matmul(
                        wu_p[:], wu_w[:], wu_v[:], start=True, stop=True
                    )

            def mm_H(psum, w_pair, rhs_pair, m, extra=None):
                """psum[m] += W @ rhs for a [H,H] weight given as 2 k-tiles."""
                first = extra is None
                if extra is not None:
                    nc.tensor.matmul(
                        psum[:], extra[0][:, m * HH : (m + 1) * HH], extra[1][:],
                        start=True, stop=False,
                    )
                nc.tensor.matmul(
                    psum[:], w_pair[0][:, m * HH : (m + 1) * HH], rhs_pair[0][:],
                    start=first, stop=False,
                )
                nc.tensor.matmul(
                    psum[:], w_pair[1][:, m * HH : (m + 1) * HH], rhs_pair[1][:],
                    start=False, stop=True,
                )

            import contextlib
            loop_cm = tc.For_i(0, loop, 1) if loop else contextlib.nullcontext()
            with loop_cm:
             for _rep in range(repeat):
              # ---- phase 1: l1 = wx@xT + wh@hT + b0; relu & keep l1 for gate ----
              l1_sb = [apool.tile([HH, BL], f32, tag=f"l1_{m}", name=f"l1_{m}") for m in range(2)]
              relu_sb = [apool.tile([HH, BL], MM_DT, tag=f"relu{m}", name=f"relu{m}") for m in range(2)]
              for m in range(2):
                  p = ps.tile([HH, BL], f32, tag="ps")
                  mm_H(p, whF, h_sb, m, extra=(wxF, x_sb))
                  nc.scalar.activation(
                      l1_sb[m][:], p[:], _ACT.Identity, bias=b0_sb[:, m : m + 1]
                  )
                  nc.scalar.activation(
                      relu_sb[m][:], p[:], _ACT.Relu, bias=b0_sb[:, m : m + 1]
                  )

              # ---- phase 2: lout = wout@relu + b1; tanh; dtanh = 1-tanh^2 ----
              dtanh_sb = [apool.tile([HH, BL], f32, tag=f"dt{m}", name=f"dt{m}") for m in range(2)]
              for m in range(2):
                  p = ps.tile([HH, BL], f32, tag="ps")
                  mm_H(p, wo_sb, relu_sb, m)
                  tanh = apool.tile([HH, BL], f32, tag=f"tanh{m}")
                  nc.scalar.activation(
                      tanh[:], p[:], _ACT.Tanh, bias=b1_sb[:, m : m + 1]
                  )
                  nc.vector.tensor_mul(dtanh_sb[m][:], tanh[:], tanh[:])
                  nc.vector.tensor_scalar(
                      dtanh_sb[m][:], dtanh_sb[m][:], -1.0, 1.0, _ALU.mult, _ALU.add
                  )

              # ---- phase 3: jx = dtanh ∘ (wout @ (drelu ∘ (wx @ xdot))) ----
              g_sb = [tgp.tile([HH, BL], MM_DT, tag=f"tg{m}", name=f"g{m}") for m in range(2)]
              for m in range(2):
                  p = ps.tile([HH, BL], f32, tag="ps")
                  nc.tensor.matmul(
                      p[:], wx_sb[:, m * HH : (m + 1) * HH], xd_sb[:],
                      start=True, stop=True,
                  )
                  # g = (l1 > 0) * u   -- fused relu-derivative gate
                  nc.vector.scalar_tensor_tensor(
                      g_sb[m][:], l1_sb[m][:], 0.0, p[:], _ALU.is_gt, _ALU.mult
                  )
              curr = [None, None]
              hdot = [apool.tile([HH, BL], f32, tag=f"hd{m}", name=f"hd{m}") for m in range(2)]
              for m in range(2):
                  p = ps.tile([HH, BL], f32, tag="ps")
                  mm_H(p, wo_sb, g_sb, m)
                  c = rot.tile([HH, BL], MM_DT, tag=f"curr{m}", name=f"curr{m}")
                  nc.vector.tensor_mul(c[:], dtanh_sb[m][:], p[:])
                  curr[m] = c
                  # h_dot starts as jx (ACT engine, off the critical path)
                  nc.scalar.copy(hdot[m][:], c[:].bitcast(f32))

              # ---- phase 4: 8 iterations of curr <- M curr; hdot += curr ----
              def mm_H_kouter(psums, w_pair, rhs_pair):
                  # k-outer order: the first two matmuls only need rhs_pair[0],
                  # so they start as soon as the k=0 half of the rhs lands.
                  for k in range(2):
                      for m in range(2):
                          nc.tensor.matmul(
                              psums[m][:],
                              w_pair[k][:, m * HH : (m + 1) * HH],
                              rhs_pair[k][:],
                              start=(k == 0), stop=(k == 1),
                          )

              for _ in range(K_TERMS):
                  tg = [tgp.tile([HH, BL], MM_DT, tag=f"tg{m}", name=f"tg{m}") for m in range(2)]
                  pt = [ps.tile([HH, BL], f32, tag="ps", name=f"pt{m}") for m in range(2)]
                  mm_H_kouter(pt, wh_sb, curr)
                  for m in range(2):
                      nc.vector.scalar_tensor_tensor(
                          tg[m][:], l1_sb[m][:], 0.0, pt[m][:], _ALU.is_gt, _ALU.mult
                      )
                  newc = [None, None]
                  pso = [ps.tile([HH, BL], f32, tag="ps", name=f"pso{m}") for m in range(2)]
                  mm_H_kouter(pso, wo_sb, tg)
                  for m in range(2):
                      c = rot.tile([HH, BL], MM_DT, tag=f"curr{m}", name=f"curr{m}")
                      nc.vector.tensor_mul(c[:], dtanh_sb[m][:], pso[m][:])
                      newc[m] = c
                      # accumulate on GpSimd so DVE stays on the critical path
                      nc.gpsimd.tensor_add(
                          hdot[m][:], hdot[m][:], c[:].bitcast(f32)
                      )
                  curr = newc

              for m in range(2):
                  nc.sync.dma_start(hdT[m * HH : (m + 1) * HH, :], hdot[m][:])

    nc.compile()
    return nc


_NC = {}


def _get_nc(repeat=1, loop=0):
    key = (repeat, loop)
    if key not in _NC:
        _NC[key] = _build(repeat, loop)
    return _NC[key]


def kernel(h, x, xdot, wx, wh, wout, b0, b1):
    h = np.asarray(h, np.float32)
    x = np.asarray(x, np.float32)
    xdot = np.asarray(xdot, np.float32)
    wx = np.asarray(wx, np.float32)
    wh = np.asarray(wh, np.float32)
    wout = np.asarray(wout, np.float32)
    b0 = np.asarray(b0, np.float32)
    b1 = np.asarray(b1, np.float32)

    whT = np.ascontiguousarray(wh.T)
    woT = np.ascontiguousarray(wout.T)
    wxT = np.ascontiguousarray(wx.T)
    b0c = np.ascontiguousarray(np.stack([b0[:HH], b0[HH:]], axis=1))
    b1c = np.ascontiguousarray(np.stack([b1[:HH], b1[HH:]], axis=1))

    in_maps = []
    for i in range(N_CORES):
        sl = slice(i * BL, (i + 1) * BL)
        in_maps.append(
            {
                "hT": np.ascontiguousarray(h[sl].T),
                "xT": np.ascontiguousarray(x[sl].T),
                "xdT": np.ascontiguousarray(xdot[sl].T),
                "wxT": wxT,
                "whT": whT,
                "woT": woT,
                "b0c": b0c,
                "b1c": b1c,
            }
        )

    res = run_bass_kernel_spmd(_get_nc(), in_maps, core_ids=list(range(N_CORES)))
    out = np.empty((B, H), np.float32)
    for i in range(N_CORES):
        out[i * BL : (i + 1) * BL] = res.results[i]["hdT"].T
    return out



# revision 7
# speedup vs baseline: 1.7754x; 1.7754x over previous
"""Trainium2 Bass kernel for the JaCDE dense-MLP vector-field problem.

Math: the reference contracts a materialized per-sample Jacobian (O(B*H^3)).
With D_r = diag(relu'(l1)), D_t = diag(1-tanh(lout)^2) fixed per sample, the
whole computation is a geometric series of the operator
    M v = D_t (Wo (D_r (Wh v)))
Let t_0 = D_r (Wx xdot) and t_k = D_r (Wh (D_t (Wo t_{k-1}))).  Then
    h_dot = sum_{k=0..K} M^k jx = D_t (Wo (sum_{k=0..K} t_k))
so only ONE D_t/Wo application is needed at the end, and the running sum
S = sum t_k accumulates for free in a dedicated PSUM bank via identity
matmuls (PE), keeping DVE/ACT off the accumulation.

Precision: phase 1 (l1 -> relu mask) runs f32r (mask flips are catastrophic);
everything after runs bf16 (measured end-to-end rel err ~5e-3 vs 2e-2 gate).
bf16 makes matmuls 1 cyc/row at any free-dim size and doubles DVE throughput
for SBUF-resident elementwise ops.

Parallel structure: batch 2048 -> 8 cores x 256; per core the 256 batch cols
split into NS=2 independent streams so the serial chain of one stream fills
the dependency bubbles of the other.  The two H-halves of each activation
live side by side in one [128, 256] tile, halving elementwise op count.
PSUM->SBUF moves alternate between DVE (direct, fused mask) and ACT-copy +
cheap bf16 DVE op, balancing the two engines.

Sharding: pure data parallel, batch 2048 -> 8 cores x 256.
"""

import numpy as np
import ml_dtypes

import concourse.tile as tile
from concourse import bacc, mybir
from concourse.bass_utils import run_bass_kernel_spmd

B, H, IN = 2048, 256, 64
K_TERMS = 8
N_CORES = 8
BL = B // N_CORES  # 256 batch rows per core
HH = H // 2  # 128, H partition halves
W = 128  # batch columns per stream
NS = BL // W  # 2 streams

f32 = mybir.dt.float32
f32r = mybir.dt.float32r
bf16 = mybir.dt.bfloat16
N_WARMUP_MM = 16

_ALU = mybir.AluOpType
_ACT = mybir.ActivationFunctionType


def _build(repeat=1, loop=0, k_terms=K_TERMS,
           gate_act=(False, True), mul_act=(True, False)):
    nc = bacc.Bacc(None, target_bir_lowering=False)

    hT = nc.dram_tensor("hT", [H, BL], f32, kind="ExternalInput")
    xT = nc.dram_tensor("xT", [IN, BL], f32, kind="ExternalInput")
    xdT = nc.dram_tensor("xdT", [IN, BL], f32r, kind="ExternalInput")
    wxT = nc.dram_tensor("wxT", [IN, H], f32, kind="ExternalInput")
    whT = nc.dram_tensor("whT", [H, H], f32, kind="ExternalInput")
    woT = nc.dram_tensor("woT", [H, H], bf16, kind="ExternalInput")
    eyeT = nc.dram_tensor("eyeT", [HH, HH], bf16, kind="ExternalInput")
    b0c = nc.dram_tensor("b0c", [HH, 2], f32, kind="ExternalInput")
    b1c = nc.dram_tensor("b1c", [HH, 2], f32, kind="ExternalInput")
    hdT = nc.dram_tensor("hdT", [H, BL], f32, kind="ExternalOutput")

    with tile.TileContext(nc) as tc:
        with (
            tc.tile_pool(name="wpool", bufs=1) as wpool,
            tc.tile_pool(name="tgp", bufs=3) as tgp,
            tc.tile_pool(name="ps", bufs=6, space="PSUM") as ps,
            tc.tile_pool(name="pacc", bufs=1, space="PSUM") as pk,
        ):
            # ---- weights / inputs to SBUF (outside the timed loop) ----
            whF = [wpool.tile([HH, H], f32, tag=f"whF{k}", name=f"whF{k}") for k in range(2)]
            wh_b = [wpool.tile([HH, H], bf16, tag=f"whb{k}", name=f"whb{k}") for k in range(2)]
            wo_b = [wpool.tile([HH, H], bf16, tag=f"wob{k}", name=f"wob{k}") for k in range(2)]
            wxF = wpool.tile([IN, H], f32, tag="wxF")
            wxR = wpool.tile([IN, H], f32r, tag="wxR")
            h_sb = [wpool.tile([HH, BL], f32, tag=f"h{k}", name=f"h{k}") for k in range(2)]
            x_sb = wpool.tile([IN, BL], f32, tag="x")
            xd_sb = wpool.tile([IN, BL], f32r, tag="xd")
            eye_b = wpool.tile([HH, HH], bf16, tag="eye")
            b0_sb = wpool.tile([HH, 2], f32, tag="b0")
            b1_sb = wpool.tile([HH, 2], f32, tag="b1")
            for k in range(2):
                nc.sync.dma_start(whF[k][:], whT[k * HH:(k + 1) * HH, :])
                nc.sync.dma_start(wo_b[k][:], woT[k * HH:(k + 1) * HH, :])
                nc.sync.dma_start(h_sb[k][:], hT[k * HH:(k + 1) * HH, :])
                nc.vector.tensor_copy(wh_b[k][:], whF[k][:])
            nc.sync.dma_start(wxF[:], wxT[:])
            nc.vector.tensor_copy(wxR[:], wxF[:])
            nc.sync.dma_start(x_sb[:], xT[:])
            nc.sync.dma_start(xd_sb[:], xdT[:])
            nc.sync.dma_start(eye_b[:], eyeT[:])
            nc.sync.dma_start(b0_sb[:], b0c[:])
            nc.sync.dma_start(b1_sb[:], b1c[:])

            # masks in per-stream layout: [128, m*W + b] for stream cols b
            relu_s = [wpool.tile([HH, 2 * W], bf16, tag=f"relu{s}", name=f"relu{s}") for s in range(NS)]
            dtc_s = [wpool.tile([HH, 2 * W], bf16, tag=f"dtc{s}", name=f"dtc{s}") for s in range(NS)]
            hd_s = [wpool.tile([HH, 2 * W], f32, tag=f"hd{s}", name=f"hd{s}") for s in range(NS)]

            # ---- PE warmup: open the HAM clock gate during input DMAs ----
            if N_WARMUP_MM:
                wu_w = wpool.tile([HH, HH], bf16, tag="wu_w")
                wu_v = wpool.tile([HH, BL], bf16, tag="wu_v")
                nc.vector.memset(wu_w[:].bitcast(f32), 0.0)
                nc.vector.memset(wu_v[:].bitcast(f32), 0.0)
                wu_p = ps.tile([HH, BL], f32, tag="ps")
                for _ in range(N_WARMUP_MM):
                    nc.tensor.matmul(wu_p[:], wu_w[:], wu_v[:], start=True, stop=True)

            import contextlib
            loop_cm = tc.For_i(0, loop, 1) if loop else contextlib.nullcontext()
            with loop_cm:
             for _rep in range(repeat):
              # ---- phase 1: l1 = wx@x + wh@h + b0 (f32r); relu mask bf16 ----
              for m in range(2):
                  ms = slice(m * HH, (m + 1) * HH)
                  p = ps.tile([HH, BL], f32, tag="ps", name=f"pl1_{m}")
                  nc.tensor.matmul(p[:], wxF[:, ms], x_sb[:], start=True, stop=False)
                  nc.tensor.matmul(p[:], whF[0][:, ms], h_sb[0][:], start=False, stop=False)
                  nc.tensor.matmul(p[:], whF[1][:, ms], h_sb[1][:], start=False, stop=True)
                  for s in range(NS):
                      nc.scalar.activation(
                          relu_s[s][:, m * W:(m + 1) * W],
                          p[:, s * W:(s + 1) * W],
                          _ACT.Relu, bias=b0_sb[:, m:m + 1],
                      )

              # ---- phase 2: lout = wo@relu + b1; dtanh = 1 - tanh^2 (bf16) ----
              tn = [tgp.tile([HH, 2 * W], bf16, tag=f"tn{s}", name=f"tn{s}") for s in range(NS)]
              for m in range(2):
                  ms = slice(m * HH, (m + 1) * HH)
                  p = ps.tile([HH, BL], f32, tag="ps", name=f"plo_{m}")
                  for s in range(NS):
                      sc = slice(s * W, (s + 1) * W)
                      nc.tensor.matmul(p[:, sc], wo_b[0][:, ms], relu_s[s][:, 0:W],
                                       start=True, stop=False)
                      nc.tensor.matmul(p[:, sc], wo_b[1][:, ms], relu_s[s][:, W:2 * W],
                                       start=False, stop=True)
                  for s in range(NS):
                      nc.scalar.activation(
                          tn[s][:, m * W:(m + 1) * W],
                          p[:, s * W:(s + 1) * W],
                          _ACT.Tanh, bias=b1_sb[:, m:m + 1],
                      )
              for s in range(NS):
                  nc.vector.tensor_mul(dtc_s[s][:], tn[s][:], tn[s][:])
                  nc.vector.tensor_scalar(
                      dtc_s[s][:], dtc_s[s][:], -1.0, 1.0, _ALU.mult, _ALU.add
                  )

              # ---- phase 3: t0 = drelu o (wx @ xdot); start S accumulation ----
              pg = []
              for m in range(2):
                  ms = slice(m * HH, (m + 1) * HH)
                  p = ps.tile([HH, BL], f32, tag="ps", name=f"pg_{m}")
                  nc.tensor.matmul(p[:], wxR[:, ms], xd_sb[:], start=True, stop=True)
                  pg.append(p)
              tg = [tgp.tile([HH, 2 * W], bf16, tag=f"tg{s}", name=f"tg{s}") for s in range(NS)]
              for s in range(NS):
                  for m in range(2):
                      nc.vector.scalar_tensor_tensor(
                          tg[s][:, m * W:(m + 1) * W],
                          relu_s[s][:, m * W:(m + 1) * W], 0.0,
                          pg[m][:, s * W:(s + 1) * W],
                          _ALU.is_gt, _ALU.mult,
                      )
              pacc = [pk.tile([HH, 2 * W], f32, tag=f"pacc{s}", name=f"pacc{s}")
                       for s in range(NS)]
              for s in range(NS):
                  nc.tensor.matmul(
                      pacc[s][:], eye_b[:], tg[s][:],
                      start=True, stop=(k_terms == 0),
                  )

              # ---- loop: t_k = D_r(Wh(D_t(Wo t_{k-1}))); S += t_k ----
              for k in range(1, k_terms + 1):
                  newtg = [tgp.tile([HH, 2 * W], bf16, tag=f"tg{s}", name=f"tg{s}") for s in range(NS)]
                  for s in range(NS):
                      pv = ps.tile([HH, 2 * W], f32, tag="ps", name=f"pv{s}_{k}")
                      for m in range(2):
                          for kk in range(2):
                              nc.tensor.matmul(
                                  pv[:, m * W:(m + 1) * W],
                                  wo_b[kk][:, m * HH:(m + 1) * HH],
                                  tg[s][:, kk * W:(kk + 1) * W],
                                  start=(kk == 0), stop=(kk == 1),
                              )
                      c = tgp.tile([HH, 2 * W], bf16, tag=f"c{s}")
                      if mul_act[s]:
                          vb = tgp.tile([HH, 2 * W], bf16, tag=f"vb{s}")
                          nc.scalar.copy(vb[:], pv[:])
                          nc.vector.tensor_mul(c[:], dtc_s[s][:], vb[:])
                      else:
                          nc.vector.tensor_mul(c[:], dtc_s[s][:], pv[:])

                      pt = ps.tile([HH, 2 * W], f32, tag="ps", name=f"pt{s}_{k}")
                      for m in range(2):
                          for kk in range(2):
                              nc.tensor.matmul(
                                  pt[:, m * W:(m + 1) * W],
                                  wh_b[kk][:, m * HH:(m + 1) * HH],
                                  c[:, kk * W:(kk + 1) * W],
                                  start=(kk == 0), stop=(kk == 1),
                              )
                      if gate_act[s]:
                          ub = tgp.tile([HH, 2 * W], bf16, tag=f"ub{s}")
                          nc.scalar.copy(ub[:], pt[:])
                          nc.vector.scalar_tensor_tensor(
                              newtg[s][:], relu_s[s][:], 0.0, ub[:],
                              _ALU.is_gt, _ALU.mult,
                          )
                      else:
                          nc.vector.scalar_tensor_tensor(
                              newtg[s][:], relu_s[s][:], 0.0, pt[:],
                              _ALU.is_gt, _ALU.mult,
                          )
                      nc.tensor.matmul(
                          pacc[s][:], eye_b[:], newtg[s][:],
                          start=False, stop=(k == k_terms),
                      )
                  tg = newtg

              # ---- epilogue: h_dot = D_t (Wo S) ----
              for s in range(NS):
                  sb = tgp.tile([HH, 2 * W], bf16, tag=f"sb{s}")
                  nc.scalar.copy(sb[:], pacc[s][:])
                  pf = ps.tile([HH, 2 * W], f32, tag="ps", name=f"pf{s}")
                  for m in range(2):
                      for kk in range(2):
                          nc.tensor.matmul(
                              pf[:, m * W:(m + 1) * W],
                              wo_b[kk][:, m * HH:(m + 1) * HH],
                              sb[:, kk * W:(kk + 1) * W],
                              start=(kk == 0), stop=(kk == 1),
                          )
                  nc.vector.tensor_mul(hd_s[s][:], dtc_s[s][:], pf[:])
                  for m in range(2):
                      nc.sync.dma_start(
                          hdT[m * HH:(m + 1) * HH, s * W:(s + 1) * W],
                          hd_s[s][:, m * W:(m + 1) * W],
                      )

    nc.compile()
    return nc


_NC = {}


def _get_nc(repeat=1, loop=0, **kw):
    key = (repeat, loop, tuple(sorted(kw.items())))
    if key not in _NC:
        _NC[key] = _build(repeat, loop, **kw)
    return _NC[key]


_EYE = np.eye(HH, dtype=ml_dtypes.bfloat16)


def make_in_maps_full(h, x, xdot, wx, wh, wout, b0, b1):
    whT = np.ascontiguousarray(wh.T)
    woT = np.ascontiguousarray(wout.T.astype(ml_dtypes.bfloat16))
    wxT = np.ascontiguousarray(wx.T)
    b0c = np.ascontiguousarray(np.stack([b0[:HH], b0[HH:]], axis=1))
    b1c = np.ascontiguousarray(np.stack([b1[:HH], b1[HH:]], axis=1))
    in_maps = []
    for i in range(N_CORES):
        sl = slice(i * BL, (i + 1) * BL)
        in_maps.append(
            {
                "hT": np.ascontiguousarray(h[sl].T),
                "xT": np.ascontiguousarray(x[sl].T),
                "xdT": np.ascontiguousarray(xdot[sl].T),
                "wxT": wxT,
                "whT": whT,
                "woT": woT,
                "eyeT": _EYE,
                "b0c": b0c,
                "b1c": b1c,
            }
        )
    return in_maps


def kernel(h, x, xdot, wx, wh, wout, b0, b1):
    h = np.asarray(h, np.float32)
    x = np.asarray(x, np.float32)
    xdot = np.asarray(xdot, np.float32)
    wx = np.asarray(wx, np.float32)
    wh = np.asarray(wh, np.float32)
    wout = np.asarray(wout, np.float32)
    b0 = np.asarray(b0, np.float32)
    b1 = np.asarray(b1, np.float32)

    in_maps = make_in_maps_full(h, x, xdot, wx, wh, wout, b0, b1)
    res = run_bass_kernel_spmd(_get_nc(), in_maps, core_ids=list(range(N_CORES)))
    out = np.empty((B, H), np.float32)
    for i in range(N_CORES):
        out[i * BL:(i + 1) * BL] = res.results[i]["hdT"].T
    return out
